# revision 1
# baseline (speedup 1.0000x reference)
"""BloomAttention (B=1, S=2048, H=4096, NH=32) on 8 Trainium2 cores.

Megatron-style tensor parallelism over heads: each core owns 4 heads.
 - QKV projection: column-parallel (each core computes its heads' Q/K/V)
 - attention: fully local per core (head-parallel)
 - dense projection: row-parallel -> per-core partial outputs, summed on host

All matmuls run in float32r (TF32-like, full PE speed at moving dim >=256).
Layouts keep the contraction dim on SBUF partitions:
   hiddenT [H, S], w_qkvT [H, 3*512], QT/KT/VT per head [128, S],
   probsT [keys, q], ctxT [128, S], w_denseT [512, H].
Causal structure is exploited by truncating each 128-query block's key range;
the diagonal 128x128 block is masked with a host-provided additive tile.
"""
import math
import numpy as np
from contextlib import ExitStack

import concourse.bacc as bacc
import concourse.bass as bass
import concourse.mybir as mybir
import concourse.tile as tile
from concourse.bass_utils import run_bass_kernel_spmd

# problem dims (hardcoded per contract)
B, S, H, NH = 1, 2048, 4096, 32
HD = H // NH            # 128
NCORES = 8
HPC = NH // NCORES      # 4 heads per core
DPC = HPC * HD          # 512 features per core
FC = 3 * HPC            # 12 feature chunks of 128 in QKV output
INV_NORM = 1.0 / math.sqrt(HD)
NEG = float(np.finfo(np.float32).min)
P = 128
QB = S // P             # 16 query blocks
F32 = mybir.dt.float32
F32R = mybir.dt.float32r

_CACHE = {}


def _build(kNq):
    """Build the SPMD program for one core. kNq[qb] = key columns needed for
    query block qb (multiple of 128). Returns compiled Bacc."""
    nc = bacc.Bacc("TRN2", target_bir_lowering=False, debug=False,
                   num_devices=NCORES)

    hT = nc.dram_tensor("hT", [H, S], F32R, kind="ExternalInput")
    wqkvT = nc.dram_tensor("wqkvT", [H, FC * P], F32R, kind="ExternalInput")
    bqkv = nc.dram_tensor("bqkv", [FC, P], F32, kind="ExternalInput")
    alibi_t = nc.dram_tensor("alibi_c", [HPC, S], F32, kind="ExternalInput")
    tri_t = nc.dram_tensor("tri", [QB, P, P], F32, kind="ExternalInput")
    ident_t = nc.dram_tensor("ident", [P, P], F32R, kind="ExternalInput")
    wdT = nc.dram_tensor("wdT", [DPC, H], F32R, kind="ExternalInput")
    ctx_sp = nc.dram_tensor("ctx_spill", [HPC, P, S], F32R)
    out_t = nc.dram_tensor("out_part", [S, H], F32, kind="ExternalOutput")

    KP = 8                      # contraction panels of 512 rows
    JP = H // KP // P           # 4 h-chunks per panel

    with tile.TileContext(nc) as tc, ExitStack() as top:
        singles = top.enter_context(tc.tile_pool(name="singles", bufs=1))
        ph12 = top.enter_context(ExitStack())
        qkv_pool = ph12.enter_context(tc.tile_pool(name="qkv", bufs=1))
        # persistent QT/KT tiles per head [128, S] (head dim on partitions)
        qk_tiles = [[qkv_pool.tile([P, S], F32R, tag=f"qkv_{c}_{h}",
                                   name=f"qkv_{c}_{h}")
                     for h in range(HPC)] for c in range(2)]
        # V in natural layout: per key-tile sc -> [128 keys, 512 hd]
        v_tiles = [qkv_pool.tile([P, DPC], F32R, tag=f"v_{sc}",
                                 name=f"v_{sc}")
                   for sc in range(S // P)]
        ident_sb = singles.tile([P, P], F32R, tag="ident")
        nc.sync.dma_start(out=ident_sb, in_=ident_t[:, :])
        bias_sb = singles.tile([P, FC], F32, tag="bias")
        nc.sync.dma_start(
            out=bias_sb,
            in_=bass.AP(tensor=bqkv, offset=0, ap=[[1, P], [P, FC]]))
        # V bias broadcast to all partitions: bqkv rows 8..11 flattened [512]
        bv_bc = singles.tile([P, DPC], F32, tag="bv_bc")
        nc.gpsimd.dma_start(
            out=bv_bc,
            in_=bass.AP(tensor=bqkv, offset=2 * HPC * P,
                        ap=[[0, P], [1, DPC]]))

        # ---------------- phase 1: QKV projection ----------------
        with ExitStack() as ph1:
            hid_pool = ph1.enter_context(tc.tile_pool(name="hid", bufs=2))
            wq_pool = ph1.enter_context(tc.tile_pool(name="wq", bufs=3))
            ps1 = ph1.enter_context(
                tc.tile_pool(name="ps1", bufs=4, space="PSUM"))
            for kp in range(KP):
                hp = hid_pool.tile([P, JP, S], F32R, tag="hp")
                for j in range(JP):
                    r0 = (kp * JP + j) * P
                    nc.sync.dma_start(out=hp[:, j, :], in_=hT[r0:r0 + P, :])
                # Q and K: feature chunks on partitions
                for fc in range(2 * HPC):
                    wt = wq_pool.tile([P, JP, P], F32R, tag="wt")
                    nc.sync.dma_start(
                        out=wt,
                        in_=wqkvT[kp * JP * P:(kp + 1) * JP * P,
                                  fc * P:(fc + 1) * P].rearrange(
                                      "(j p) f -> p j f", p=P))
                    comp, head = fc // HPC, fc % HPC
                    dest = qk_tiles[comp][head]
                    for sb2 in range(S // 1024):
                        ps = ps1.tile([P, 1024], F32, tag="ps1")
                        for half in range(2):
                            for j in range(JP):
                                nc.tensor.matmul(
                                    ps[:, half * 512:(half + 1) * 512],
                                    wt[:, j, :],
                                    hp[:, j, sb2 * 1024 + half * 512:
                                       sb2 * 1024 + (half + 1) * 512],
                                    start=(j == 0), stop=(j == JP - 1))
                        dsl = dest[:, sb2 * 1024:(sb2 + 1) * 1024]
                        if kp == 0:
                            nc.scalar.activation(
                                out=dsl, in_=ps,
                                func=mybir.ActivationFunctionType.Identity,
                                bias=bias_sb[:, fc:fc + 1], scale=1.0)
                        else:
                            nc.vector.tensor_add(
                                out=dsl, in0=ps, in1=dsl)
                # V: natural layout, hidden chunks stationary, wv moving
                wv = wq_pool.tile([P, JP, DPC], F32R, tag="wv")
                nc.sync.dma_start(
                    out=wv,
                    in_=wqkvT[kp * JP * P:(kp + 1) * JP * P,
                              2 * HPC * P:].rearrange(
                                  "(j p) f -> p j f", p=P))
                for sc2 in range(S // P // 2):
                    ps = ps1.tile([P, 1024], F32, tag="ps1")
                    for half in range(2):
                        sc = 2 * sc2 + half
                        for j in range(JP):
                            nc.tensor.matmul(
                                ps[:, half * 512:(half + 1) * 512],
                                hp[:, j, sc * P:(sc + 1) * P],
                                wv[:, j, :],
                                start=(j == 0), stop=(j == JP - 1))
                    # drain both halves; v tiles are per key-tile [128, 512]
                    for half in range(2):
                        sc = 2 * sc2 + half
                        psl = ps[:, half * 512:(half + 1) * 512]
                        if kp == 0:
                            nc.scalar.copy(out=v_tiles[sc], in_=psl)
                        else:
                            nc.vector.tensor_add(
                                out=v_tiles[sc], in0=psl, in1=v_tiles[sc])
                if kp == KP - 1:
                    # fold in the V bias (broadcast row over partitions)
                    for sc in range(S // P):
                        nc.vector.tensor_add(
                            out=v_tiles[sc], in0=v_tiles[sc], in1=bv_bc)

        # ---------------- phase 2: attention ----------------
        with ExitStack() as ph2:
            al_pool = ph2.enter_context(tc.tile_pool(name="alibi", bufs=2))
            sr_pool = ph2.enter_context(tc.tile_pool(name="srow", bufs=3))
            pr_pool = ph2.enter_context(tc.tile_pool(name="prow", bufs=2))
            pq_pool = ph2.enter_context(tc.tile_pool(name="pquad", bufs=4))
            tri_pool = ph2.enter_context(tc.tile_pool(name="tri", bufs=2))
            sm_pool = ph2.enter_context(tc.tile_pool(name="small", bufs=8))
            cs_pool = ph2.enter_context(tc.tile_pool(name="ctxstage", bufs=2))
            ps_sc = ph2.enter_context(
                tc.tile_pool(name="ps_sc", bufs=3, space="PSUM"))
            ps_st = ph2.enter_context(
                tc.tile_pool(name="ps_st", bufs=3, space="PSUM"))
            ps_cx = ph2.enter_context(
                tc.tile_pool(name="ps_cx", bufs=2, space="PSUM"))

            NG = QB // 4        # 4 query groups of 512
            for h in range(HPC):
                qt, kt = qk_tiles[0][h], qk_tiles[1][h]
                alb = al_pool.tile([P, S], F32, tag="alb")
                nc.gpsimd.dma_start(
                    out=alb,
                    in_=bass.AP(tensor=alibi_t, offset=h * S,
                                ap=[[0, P], [1, S]]))

                for qg in range(NG):
                    kns = [kNq[4 * qg + i] for i in range(4)]
                    ntile = max(kns) // P
                    nquad = (ntile + 3) // 4
                    quads = [pq_pool.tile([P, 4, 512], F32R, tag="pquad",
                                          name=f"pq_{h}_{qg}_{a}")
                             for a in range(nquad)]

                    for qbl in range(4):
                        qb = 4 * qg + qbl
                        kN = kNq[qb]
                        nch = (kN + 511) // 512
                        srow = sr_pool.tile([P, S], F32, tag="srow")
                        prow = pr_pool.tile([P, S], F32R, tag="prow")
                        nbias = sm_pool.tile([P, 1], F32, tag="nbias")
                        tot = sm_pool.tile([P, 1], F32, tag="tot")
                        rinv = sm_pool.tile([P, 1], F32, tag="rinv")
                        tri_sb = tri_pool.tile([P, P], F32, tag="tri")
                        nc.sync.dma_start(out=tri_sb, in_=tri_t[qb])
                        for kc in range(nch):
                            N = min(512, kN - 512 * kc)
                            ps = ps_sc.tile([P, 512], F32, tag="ps_sc")
                            nc.tensor.matmul(
                                ps[:, :N], qt[:, qb * P:(qb + 1) * P],
                                kt[:, kc * 512:kc * 512 + N],
                                start=True, stop=True)
                            # scores + alibi -> SBUF (frees the PSUM bank)
                            nc.vector.tensor_add(
                                out=srow[:, kc * 512:kc * 512 + N],
                                in0=ps[:, :N],
                                in1=alb[:, kc * 512:kc * 512 + N])
                        nc.vector.tensor_add(
                            out=srow[:, kN - P:kN], in0=srow[:, kN - P:kN],
                            in1=tri_sb)
                        nc.vector.tensor_reduce(
                            out=nbias, in_=srow[:, :kN],
                            op=mybir.AluOpType.max,
                            axis=mybir.AxisListType.X, negate=True)
                        nc.scalar.activation(
                            out=prow[:, :kN], in_=srow[:, :kN],
                            func=mybir.ActivationFunctionType.Exp,
                            bias=nbias, scale=1.0, accum_out=tot)
                        nc.vector.reciprocal(out=rinv, in_=tot)
                        nc.vector.tensor_scalar_mul(
                            out=prow[:, :kN], in0=prow[:, :kN], scalar1=rinv)
                        # transpose probs into key-major quads
                        # quad tile layout: [P, tile_in_quad(4), q(512)]
                        ntile_q = kN // P
                        t = 0
                        while t < ntile_q:
                            cnt = min(4, ntile_q - t)
                            stg = ps_st.tile([P, 4, P], F32R, tag="stg")
                            for j in range(cnt):
                                nc.tensor.transpose(
                                    stg[:, j, :],
                                    prow[:, (t + j) * P:(t + j + 1) * P],
                                    ident_sb)
                            nc.scalar.copy(
                                out=quads[t // 4][:, :cnt,
                                                  qbl * P:(qbl + 1) * P],
                                in_=stg[:, :cnt, :])
                            t += cnt
                    # PV: ctxT[128, 512] accumulate over key tiles.
                    # Ragged key tiles only contribute to the query sub-blocks
                    # that cover them -> accumulate into a column slice; tile 0
                    # is covered by every sub-block, so start=True initializes
                    # the full bank.
                    tiles_per_qbl = [kns[i] // P for i in range(4)]
                    cps = ps_cx.tile([P, 512], F32, tag="ps_cx")
                    for t in range(ntile):
                        q0 = P * min(i for i in range(4)
                                     if tiles_per_qbl[i] > t)
                        nc.tensor.matmul(
                            cps[:, q0:], v_tiles[t][:, h * P:(h + 1) * P],
                            quads[t // 4][:, t % 4, q0:],
                            start=(t == 0), stop=(t == ntile - 1))
                    cst = cs_pool.tile([P, 512], F32R, tag="cst")
                    nc.scalar.copy(out=cst, in_=cps)
                    nc.sync.dma_start(
                        out=ctx_sp[h, :, qg * 512:(qg + 1) * 512], in_=cst)

        ph12.close()  # free QKV + attention SBUF before dense phase

        # ---------------- phase 3: dense projection ----------------
        with ExitStack() as ph3:
            cx_pool = ph3.enter_context(tc.tile_pool(name="cx", bufs=1))
            wd_pool = ph3.enter_context(tc.tile_pool(name="wd", bufs=1))
            st_pool = ph3.enter_context(tc.tile_pool(name="ostage", bufs=2))
            ps3 = ph3.enter_context(
                tc.tile_pool(name="ps3", bufs=8, space="PSUM"))
            ctx_sb = [cx_pool.tile([P, S], F32R, tag=f"ctx_{hh}", name=f"ctx_{hh}")
                      for hh in range(HPC)]
            for hh in range(HPC):
                for sc in range(4):
                    nc.sync.dma_start(
                        out=ctx_sb[hh][:, sc * 512:(sc + 1) * 512],
                        in_=ctx_sp[hh, :, sc * 512:(sc + 1) * 512])
            wdt = wd_pool.tile([P, HPC, H], F32R, tag="wdt")
            for oc in range(8):
                for hh in range(HPC):
                    nc.sync.dma_start(
                        out=wdt[:, hh, oc * 512:(oc + 1) * 512],
                        in_=wdT[hh * P:(hh + 1) * P, oc * 512:(oc + 1) * 512])
            for qb in range(QB):
                pss = [ps3.tile([P, 512], F32, tag="ps3", name=f"ps3_{qb}_{i}")
                       for i in range(8)]
                for oc in range(8):
                    for dc in range(HPC):
                        nc.tensor.matmul(
                            pss[oc], ctx_sb[dc][:, qb * P:(qb + 1) * P],
                            wdt[:, dc, oc * 512:(oc + 1) * 512],
                            start=(dc == 0), stop=(dc == HPC - 1))
                stage = st_pool.tile([P, H], F32, tag="ostage")
                for oc in range(8):
                    nc.any.tensor_copy(
                        out=stage[:, oc * 512:(oc + 1) * 512], in_=pss[oc])
                nc.sync.dma_start(
                    out=out_t[qb * P:(qb + 1) * P, :], in_=stage)

    nc.compile()
    return nc


def _host_prep(hidden_states, alibi, attention_mask, w_qkv, b_qkv, w_dense):
    """Returns (kNq, in_maps) for the 8 cores."""
    hidden = np.asarray(hidden_states, np.float32).reshape(S, H)
    mask = np.asarray(attention_mask).reshape(S, S)
    alibi = np.asarray(alibi, np.float32).reshape(NH, S)
    w_qkv = np.asarray(w_qkv, np.float32)
    b_qkv = np.asarray(b_qkv, np.float32)
    w_dense = np.asarray(w_dense, np.float32)

    allowed = ~mask
    assert allowed.any(axis=1).all(), "fully-masked row"
    limit = S - np.argmax(allowed[:, ::-1], axis=1)      # last allowed + 1
    recon = np.arange(S)[None, :] >= limit[:, None]
    if not np.array_equal(mask, recon):
        raise NotImplementedError("mask is not suffix-structured")
    kNq, tri = [], np.zeros((QB, P, P), np.float32)
    col = np.arange(S)
    for qb in range(QB):
        lb = limit[qb * P:(qb + 1) * P]
        kN = int(math.ceil(lb.max() / P) * P)
        if lb.min() < kN - P:
            raise NotImplementedError("mask boundary spans >128 cols in block")
        kNq.append(kN)
        cc = col[kN - P:kN]
        tri[qb] = np.where(cc[None, :] >= lb[:, None], NEG, 0.0)
    if any(kNq[i] > kNq[i + 1] for i in range(QB - 1)):
        raise NotImplementedError("non-monotone key ranges")

    hT = np.ascontiguousarray(hidden.T)                  # [H, S]
    wr = w_qkv.reshape(NH, 3, HD, H)
    br = b_qkv.reshape(NH, 3, HD)
    ident = np.eye(P, dtype=np.float32)

    in_maps = []
    for c in range(NCORES):
        hs = slice(HPC * c, HPC * (c + 1))
        Wq = wr[hs, 0].reshape(DPC, H) * INV_NORM
        Wk = wr[hs, 1].reshape(DPC, H)
        Wv = wr[hs, 2].reshape(DPC, H)
        wqkvT_c = np.ascontiguousarray(
            np.concatenate([Wq, Wk, Wv], axis=0).T)      # [H, 1536]
        bq = br[hs, 0].reshape(-1) * INV_NORM
        bk = br[hs, 1].reshape(-1)
        bv = br[hs, 2].reshape(-1)
        bqkv_c = np.concatenate([bq, bk, bv]).reshape(FC, P)
        wdT_c = np.ascontiguousarray(
            w_dense[:, DPC * c:DPC * (c + 1)].T)         # [512, H]
        in_maps.append({
            "hT": hT, "wqkvT": wqkvT_c, "bqkv": bqkv_c,
            "alibi_c": np.ascontiguousarray(alibi[hs]),
            "tri": tri, "ident": ident, "wdT": wdT_c,
        })
    return tuple(kNq), in_maps


def kernel(hidden_states, residual, alibi, attention_mask,
           w_qkv, b_qkv, w_dense, b_dense):
    kNq, in_maps = _host_prep(hidden_states, alibi, attention_mask,
                              w_qkv, b_qkv, w_dense)
    if kNq not in _CACHE:
        _CACHE[kNq] = _build(kNq)
    nc = _CACHE[kNq]
    res = run_bass_kernel_spmd(nc, in_maps, list(range(NCORES)))
    acc = res.results[0]["out_part"].astype(np.float64)
    for c in range(1, NCORES):
        acc += res.results[c]["out_part"]
    out = acc.astype(np.float32) + np.asarray(b_dense, np.float32)[None, :]
    out = out + np.asarray(residual, np.float32).reshape(S, H)
    return out.reshape(B, S, H).astype(np.float32)



# revision 41
# speedup vs baseline: 1.2546x; 1.2546x over previous
"""BloomAttention (B=1, S=2048, H=4096, NH=32) on 8 Trainium2 cores — v2.

Tensor-parallel over heads, 4 heads/core, stride-8 head assignment
(core c owns global heads {c, 8+c, 16+c, 24+c}) so that the shared SPMD
attention schedule (per-slot ALiBi key windows) is load-balanced.

All matmuls in bf16 (fp32 PSUM accumulation). Key structural points:
 - QKV: weights resident in SBUF, hidden streamed in 256-col panels,
   full PSUM accumulation over the 4096-contraction -> one ACT drain.
 - scores = Q.T K in PSUM; alibi added via a 2-partition accumulating
   matmul (hi/lo bf16 split of alibi for exactness); causal diagonal
   mask via identity-lhsT matmul of an additive -30000 tile.
 - softmax: no max-reduction. exp bias = -(cummax_k alibi + C) per
   query (host-computed, exact softmax shift). ACT exp reads PSUM,
   writes bf16 probs to SBUF with accum_out giving the row sum.
 - probs transposed key-major by the DMA xbar engine
   (dma_start_transpose, bf16) -> no PE transposes, no PSUM copies.
 - ALiBi locality: keys with alibi deficit > 64 contribute < e^-46;
   per-slot key windows truncate score/exp/PV work.
 - k-bias dropped (softmax-invariant), v-bias folded into the host-side
   output add (sum p = 1), q-bias folded into the Q drain.
 - dense: row-parallel, bf16 partial outputs summed on host.
"""
import math
import numpy as np
import ml_dtypes
from contextlib import ExitStack

import concourse.bacc as bacc
import concourse.bass as bass
import concourse.mybir as mybir
import concourse.tile as tile
from concourse.bass_utils import run_bass_kernel_spmd

B, S, H, NH = 1, 2048, 4096, 32
HD = H // NH            # 128
NCORES = 8
HPC = NH // NCORES      # 4 heads (slots) per core
DPC = HPC * HD          # 512
INV_NORM = 1.0 / math.sqrt(HD)
P = 128
QB = S // P             # 16 query blocks
KPJ = H // P            # 32 contraction chunks of 128
F32 = mybir.dt.float32
BF16 = mybir.dt.bfloat16
NEGBIG = -30000.0
CSHIFT = 2.0
MARGIN = 40.0           # dropped keys contribute < e^-(40-18) ~ 3e-10

_CACHE = {}
BF = ml_dtypes.bfloat16
DEBUG_DUMP = False


def _build(t0s):
    """t0s[h][qb] = first key tile for slot h, query block qb."""
    nc = bacc.Bacc("TRN2", target_bir_lowering=False, debug=False,
                   num_devices=NCORES)

    hT = nc.dram_tensor("hT", [S // 256, P, KPJ, 256], BF16,
                        kind="ExternalInput")
    w_qk = nc.dram_tensor("w_qk", [2 * HPC, P, KPJ, P], BF16,
                          kind="ExternalInput")
    w_v = nc.dram_tensor("w_v", [P, KPJ, HPC * P], BF16,
                         kind="ExternalInput")
    bq_t = nc.dram_tensor("bq", [HPC, P], F32, kind="ExternalInput")
    al2_t = nc.dram_tensor("al2", [2, HPC, S], BF16, kind="ExternalInput")
    nbc_t = nc.dram_tensor("nbc", [HPC, QB, P], F32, kind="ExternalInput")
    tri_t = nc.dram_tensor("tri", [P, QB, P], BF16, kind="ExternalInput")
    id_t = nc.dram_tensor("ident", [P, P], BF16, kind="ExternalInput")
    wdT = nc.dram_tensor("wdT", [DPC, H], BF16, kind="ExternalInput")
    out_t = nc.dram_tensor("out_part", [H, S], BF16, kind="ExternalOutput")
    if DEBUG_DUMP:
        dbg_q = nc.dram_tensor("dbg_q", [HPC, P, S], BF16, kind="ExternalOutput")
        dbg_k = nc.dram_tensor("dbg_k", [HPC, P, S], BF16, kind="ExternalOutput")
        dbg_v = nc.dram_tensor("dbg_v", [S // P, P, DPC], BF16,
                               kind="ExternalOutput")
        dbg_ctx = nc.dram_tensor("dbg_ctx", [HPC, P, S], BF16,
                                 kind="ExternalOutput")

    SB = 256                # token panel width in QKV phase
    NSB = S // SB

    with tile.TileContext(nc) as tc, ExitStack() as top:
        singles = top.enter_context(tc.tile_pool(name="singles", bufs=1))
        # persistent QKV outputs
        qk = [[singles.tile([P, S], BF16, tag=f"qk{c}{h}", name=f"qk{c}{h}")
               for h in range(HPC)] for c in range(2)]
        v_tiles = [singles.tile([P, DPC], BF16, tag=f"v{sc}", name=f"v{sc}")
                   for sc in range(S // P)]
        bq_sb = singles.tile([P, HPC], F32, tag="bq")
        nc.gpsimd.dma_start(
            out=bq_sb, in_=bass.AP(tensor=bq_t, offset=0, ap=[[1, P], [P, HPC]]))

        # ---------------- phase 1: QKV projection ----------------
        # fc-outer / kpj-inner (one PSUM group per bank, never interleaved —
        # interleaving accumulation groups across banks mis-accumulates).
        # Weights stream per-fc so fc0's matmuls start after ~3us, not after
        # the full 12.6MB weight load.
        with ExitStack() as ph1:
            w_pool = ph1.enter_context(tc.tile_pool(name="wq", bufs=1))
            hid_pool = ph1.enter_context(tc.tile_pool(name="hid", bufs=3))
            ps_qk = ph1.enter_context(
                tc.tile_pool(name="ps_qk", bufs=5, space="PSUM"))
            ps_v = ph1.enter_context(
                tc.tile_pool(name="ps_v", bufs=3, space="PSUM"))
            hps = [hid_pool.tile([P, KPJ, SB], BF16, tag="hp", name=f"hp{sb}")
                   for sb in range(NSB)]
            # first PAIR of panels before the weight stream (the first
            # matmul pair needs both), rest after
            nc.sync.dma_start(out=hps[0], in_=hT[0])
            nc.sync.dma_start(out=hps[1], in_=hT[1])
            wqk_sb = w_pool.tile([P, 2 * HPC, KPJ, P], BF16, tag="wqk")
            for fc in range(2 * HPC):
                nc.sync.dma_start(
                    out=wqk_sb[:, fc, :, :],
                    in_=w_qk[fc].rearrange("p k c -> p (k c)"))
            wv_sb = w_pool.tile([P, KPJ, HPC * P], BF16, tag="wv")
            nc.sync.dma_start(out=wv_sb, in_=w_v.rearrange("p k c -> p (k c)"))
            for sb in range(2, NSB):
                nc.sync.dma_start(out=hps[sb], in_=hT[sb])
            # Q/K: process panels in PAIRS sharing each LDWEIGHTS — the
            # second matmul streams while the next weight chunk loads.
            for sbp in range(NSB // 2):
                hpa, hpb = hps[2 * sbp], hps[2 * sbp + 1]
                for fc in range(2 * HPC):
                    psa = ps_qk.tile([P, SB], F32, tag="ps_qk",
                                     name=f"psqa{sbp}_{fc}")
                    psb = ps_qk.tile([P, SB], F32, tag="ps_qk",
                                     name=f"psqb{sbp}_{fc}")
                    for kpj in range(KPJ):
                        w = wqk_sb[:, fc, kpj, :]
                        nc.tensor.matmul(psa, w, hpa[:, kpj, :],
                                         start=(kpj == 0),
                                         stop=(kpj == KPJ - 1))
                        nc.tensor.matmul(psb, w, hpb[:, kpj, :],
                                         start=(kpj == 0),
                                         stop=(kpj == KPJ - 1))
                    comp, hh = fc // HPC, fc % HPC
                    for half, ps in ((0, psa), (1, psb)):
                        sb = 2 * sbp + half
                        dest = qk[comp][hh][:, sb * SB:(sb + 1) * SB]
                        if comp == 0:
                            nc.scalar.activation(
                                out=dest, in_=ps,
                                func=mybir.ActivationFunctionType.Identity,
                                bias=bq_sb[:, hh:hh + 1], scale=1.0)
                        else:
                            nc.vector.tensor_copy(out=dest, in_=ps)
                for half in range(2):
                    sb = 2 * sbp + half
                    hp = hps[sb]
                    for ss in range(SB // P):
                        ps = ps_v.tile([P, DPC], F32, tag="ps_v",
                                       name=f"psv{sb}_{ss}")
                        for kpj in range(KPJ):
                            nc.tensor.matmul(
                                ps, hp[:, kpj, ss * P:(ss + 1) * P],
                                wv_sb[:, kpj, :],
                                start=(kpj == 0), stop=(kpj == KPJ - 1))
                        nc.vector.tensor_copy(
                            out=v_tiles[sb * (SB // P) + ss], in_=ps)

        # ---------------- phase 2+3 shared pools ----------------
        with ExitStack() as ph23:
            late = ph23.enter_context(tc.tile_pool(name="late", bufs=1))
            ctx = [late.tile([P, S], BF16, tag=f"ctx{h}", name=f"ctx{h}")
                   for h in range(HPC)]
            wdt = late.tile([P, HPC, H], BF16, tag="wdt")
            for dc in range(HPC):
                nc.sync.dma_start(
                    out=wdt[:, dc, :], in_=wdT[dc * P:(dc + 1) * P, :])
            al2_sb = late.tile([2, HPC, S], BF16, tag="al2")
            nc.gpsimd.dma_start(out=al2_sb, in_=al2_t[:, :, :])
            nbc_sb = [late.tile([P, QB], F32, tag=f"nbc{h}", name=f"nbc{h}")
                      for h in range(HPC)]
            for h in range(HPC):
                nc.gpsimd.dma_start(
                    out=nbc_sb[h],
                    in_=bass.AP(tensor=nbc_t, offset=h * QB * P,
                                ap=[[1, P], [P, QB]]))
            tri_sb = late.tile([P, QB, P], BF16, tag="tri")
            nc.gpsimd.dma_start(out=tri_sb, in_=tri_t[:, :, :])
            id_sb = late.tile([P, P], BF16, tag="ident")
            nc.gpsimd.dma_start(out=id_sb, in_=id_t[:, :])
            ones2 = late.tile([2, P], BF16, tag="ones2")
            nc.vector.memset(ones2, 1.0)

            if DEBUG_DUMP:
                for h in range(HPC):
                    nc.sync.dma_start(out=dbg_q[h], in_=qk[0][h])
                    nc.sync.dma_start(out=dbg_k[h], in_=qk[1][h])
                for sc in range(S // P):
                    nc.sync.dma_start(out=dbg_v[sc], in_=v_tiles[sc])

            # ---------- phase 2+3: attention with PV pipelined one step
            # behind scores, dense interleaved per query-group ----------
            with ExitStack() as ph2:
                pr_pool = ph2.enter_context(tc.tile_pool(name="prow", bufs=5))
                qd_pool = ph2.enter_context(tc.tile_pool(name="quads", bufs=2))
                sm_pool = ph2.enter_context(tc.tile_pool(name="small", bufs=8))
                ps_sc = ph2.enter_context(
                    tc.tile_pool(name="ps_sc", bufs=5, space="PSUM"))
                ps_cx = ph2.enter_context(
                    tc.tile_pool(name="ps_cx", bufs=3, space="PSUM"))

                def emit_scores(qg, h):
                    """scores+softmax chain for (qg, h); returns quads tile."""
                    qt, kt = qk[0][h], qk[1][h]
                    tmin = t0s[h][4 * qg]
                    ntg = (4 * qg + 3) - tmin + 1
                    quads = qd_pool.tile([P, ntg, 512], BF16, tag="quads",
                                         name=f"qd_{h}_{qg}")
                    for qbl in range(4):
                        qb = 4 * qg + qbl
                        t0q = t0s[h][qb]
                        twin = qb - t0q + 1
                        win = twin * P
                        koff = t0q * P
                        prow = pr_pool.tile([P, S], BF16, tag="prow",
                                            name=f"prow_{h}_{qb}")
                        tparts = sm_pool.tile([P, 4], F32, tag="tparts",
                                              name=f"tp_{h}_{qb}")
                        tot = sm_pool.tile([P, 1], F32, tag="tot",
                                           name=f"tot_{h}_{qb}")
                        rinv = sm_pool.tile([P, 1], F32, tag="rinv",
                                            name=f"ri_{h}_{qb}")
                        nch = (win + 511) // 512
                        for kc in range(nch):
                            c0 = kc * 512
                            N = min(512, win - c0)
                            has_diag = (c0 + N == win)
                            ps = ps_sc.tile([P, 512], F32, tag="ps_sc",
                                            name=f"ps_{h}_{qb}_{kc}")
                            nc.tensor.matmul(
                                ps[:, :N], qt[:, qb * P:(qb + 1) * P],
                                kt[:, koff + c0:koff + c0 + N],
                                start=True, stop=False)
                            nc.tensor.matmul(
                                ps[:, :N], ones2,
                                al2_sb[:, h, koff + c0:koff + c0 + N],
                                start=False, stop=not has_diag)
                            if has_diag:
                                nc.tensor.matmul(
                                    ps[:, N - P:N], id_sb, tri_sb[:, qb, :],
                                    start=False, stop=True)
                            nc.scalar.activation(
                                out=prow[:, c0:c0 + N], in_=ps[:, :N],
                                func=mybir.ActivationFunctionType.Exp,
                                bias=nbc_sb[h][:, qb:qb + 1], scale=1.0,
                                accum_out=tparts[:, kc:kc + 1])
                        nc.vector.tensor_reduce(
                            out=tot, in_=tparts[:, :nch],
                            op=mybir.AluOpType.add, axis=mybir.AxisListType.X)
                        nc.vector.reciprocal(out=rinv, in_=tot)
                        nc.vector.tensor_scalar_mul(
                            out=prow[:, :win], in0=prow[:, :win], scalar1=rinv)
                        nc.sync.dma_start_transpose(
                            out=quads[:, t0q - tmin:t0q - tmin + twin,
                                      qbl * P:(qbl + 1) * P],
                            in_=prow[:, :win])
                    return quads

                def emit_pv(qg, h, quads):
                    # single start per bank: first_mm clears has_written
                    # bank-wide, later first-writes overwrite (hw=0 there).
                    tmin = t0s[h][4 * qg]
                    cps = ps_cx.tile([P, 512], F32, tag="ps_cx",
                                     name=f"cps_{h}_{qg}")
                    sched = [(t, qbl) for t in range(tmin, 4 * qg + 4)
                             for qbl in range(4)
                             if t0s[h][4 * qg + qbl] <= t <= 4 * qg + qbl]
                    for i, (t, qbl) in enumerate(sched):
                        nc.tensor.matmul(
                            cps[:, qbl * P:(qbl + 1) * P],
                            v_tiles[t][:, h * P:(h + 1) * P],
                            quads[:, t - tmin, qbl * P:(qbl + 1) * P],
                            start=(i == 0), stop=(i == len(sched) - 1),
                            skip_group_check=True)
                    nc.vector.tensor_copy(
                        out=ctx[h][:, qg * 512:(qg + 1) * 512], in_=cps)

                # PV lags one (qg, h) step behind its scores so the PE runs
                # the next step's score matmuls while the softmax chain
                # (ACT exp / DVE norm / DMA transpose) completes.
                prev = None
                for qg in range(QB // 4):
                    for h in range(HPC):
                        quads = emit_scores(qg, h)
                        if prev is not None:
                            emit_pv(*prev)
                        prev = (qg, h, quads)
                emit_pv(*prev)

            if DEBUG_DUMP:
                for h in range(HPC):
                    nc.sync.dma_start(out=dbg_ctx[h], in_=ctx[h])

            # ---------------- phase 3: dense (outT layout) ----------------
            # weight-stationary: each LDWEIGHTS serves 4 N=512 matmuls
            # (moving = ctx query panels). Output is [H, S]; host transposes.
            with ExitStack() as ph3:
                st_pool = ph3.enter_context(tc.tile_pool(name="ostage", bufs=2))
                ps3 = ph3.enter_context(
                    tc.tile_pool(name="ps3", bufs=8, space="PSUM"))
                for ocb in range(H // P):
                    pqs = [ps3.tile([P, 512], F32, tag="ps3",
                                    name=f"pd_{ocb}_{qg}")
                           for qg in range(4)]
                    for dc in range(HPC):
                        for qg in range(4):
                            nc.tensor.matmul(
                                pqs[qg], wdt[:, dc, ocb * P:(ocb + 1) * P],
                                ctx[dc][:, qg * 512:(qg + 1) * 512],
                                start=(dc == 0), stop=(dc == HPC - 1),
                                skip_group_check=True)
                    stage = st_pool.tile([P, S], BF16, tag="ostage",
                                         name=f"st_{ocb}")
                    for qg in range(4):
                        if qg % 2 == 0:
                            nc.scalar.copy(
                                out=stage[:, qg * 512:(qg + 1) * 512],
                                in_=pqs[qg])
                        else:
                            nc.vector.tensor_copy(
                                out=stage[:, qg * 512:(qg + 1) * 512],
                                in_=pqs[qg])
                    nc.sync.dma_start(
                        out=out_t[ocb * P:(ocb + 1) * P, :], in_=stage)

    nc.compile()
    return nc


def _host_prep(hidden_states, alibi, attention_mask, w_qkv, b_qkv, w_dense):
    hidden = np.asarray(hidden_states, np.float32).reshape(S, H)
    mask = np.asarray(attention_mask).reshape(S, S)
    alibi = np.asarray(alibi, np.float32).reshape(NH, S)
    w_qkv = np.asarray(w_qkv, np.float32)
    b_qkv = np.asarray(b_qkv, np.float32)
    w_dense = np.asarray(w_dense, np.float32)

    allowed = ~mask
    assert allowed.any(axis=1).all(), "fully-masked row"
    limit = S - np.argmax(allowed[:, ::-1], axis=1)      # last allowed + 1
    recon = np.arange(S)[None, :] >= limit[:, None]
    if not np.array_equal(mask, recon):
        raise NotImplementedError("mask is not suffix-structured")
    col = np.arange(S)
    tri = np.zeros((QB, P, P), np.float32)
    for qb in range(QB):
        lb = limit[qb * P:(qb + 1) * P]
        kN = int(math.ceil(lb.max() / P) * P)
        if kN != (qb + 1) * P:
            raise NotImplementedError("mask is not causal-shaped")
        if lb.min() < kN - P:
            raise NotImplementedError("mask boundary spans >128 cols")
        cc = col[kN - P:kN]
        tri[qb] = np.where(cc[None, :] >= lb[:, None], NEGBIG, 0.0)

    # exp-shift: B_q = max alibi over allowed keys of q (cummax at limit-1)
    cmax = np.maximum.accumulate(alibi, axis=1)          # [NH, S]
    Bq = cmax[:, limit - 1]                              # [NH, S] per query
    negBC = -(Bq + CSHIFT)                               # [NH, S]

    # ALiBi windows: tile t droppable for (h, qb) if its max alibi is
    # MARGIN below the smallest B_q in the block.
    amax_t = alibi.reshape(NH, QB, P).max(axis=2)        # [NH, 16]
    minB = Bq.reshape(NH, QB, P).min(axis=2)             # [NH, 16]
    t0 = np.zeros((NH, QB), np.int64)
    for hgl in range(NH):
        for qb in range(QB):
            t = 0
            while t < qb and amax_t[hgl, t] <= minB[hgl, qb] - MARGIN:
                t += 1
            t0[hgl, qb] = t
    # slot schedule: union over cores, then monotone (suffix-min)
    t0s = []
    for j in range(HPC):
        tj = t0[[8 * j + c for c in range(NCORES)]].min(axis=0)
        tj = np.minimum.accumulate(tj[::-1])[::-1]
        t0s.append(tuple(int(x) for x in tj))
    t0s = tuple(t0s)

    # blocked layout for contiguous panel DMAs: [sb, p, kpj, s]
    hT = np.ascontiguousarray(
        hidden.reshape(S // 256, 256, KPJ, P).transpose(0, 3, 2, 1)
    ).astype(BF)
    wr = w_qkv.reshape(NH, 3, HD, H)
    br = b_qkv.reshape(NH, 3, HD)
    ident = np.eye(P, dtype=np.float32).astype(BF)
    tri_bf = np.ascontiguousarray(tri.transpose(1, 0, 2)).astype(BF)

    in_maps = []
    for c in range(NCORES):
        heads = [8 * j + c for j in range(HPC)]
        Wq = wr[heads, 0].reshape(DPC, H) * INV_NORM
        Wk = wr[heads, 1].reshape(DPC, H)
        Wv = wr[heads, 2].reshape(DPC, H)
        wqkT = np.concatenate([Wq, Wk], axis=0).T            # [H, 1024]
        # per-fc blocked: [8, P, KPJ, P];  V: [P, KPJ, 512]
        wqk_blk = np.ascontiguousarray(
            wqkT.reshape(H, 2 * HPC, P).transpose(1, 0, 2)
                .reshape(2 * HPC, KPJ, P, P).transpose(0, 2, 1, 3)
        ).astype(BF)
        wv_blk = np.ascontiguousarray(
            Wv.T.reshape(KPJ, P, DPC).transpose(1, 0, 2)).astype(BF)
        bq = (br[heads, 0] * INV_NORM).astype(np.float32)       # [HPC, 128]
        alc = alibi[heads]                                       # [HPC, S]
        hi = alc.astype(BF)
        lo = (alc - hi.astype(np.float32)).astype(BF)
        al2 = np.stack([hi, lo])                                 # [2,HPC,S]
        nbc = negBC[heads].reshape(HPC, QB, P).astype(np.float32)
        wd_rows = np.concatenate(
            [w_dense[:, (8 * j + c) * P:(8 * j + c + 1) * P].T
             for j in range(HPC)], axis=0)                       # [512, H]
        in_maps.append({
            "hT": hT, "w_qk": wqk_blk, "w_v": wv_blk, "bq": bq,
            "al2": np.ascontiguousarray(al2),
            "nbc": np.ascontiguousarray(nbc),
            "tri": tri_bf, "ident": ident,
            "wdT": np.ascontiguousarray(wd_rows).astype(BF),
        })
    return t0s, in_maps


def kernel(hidden_states, residual, alibi, attention_mask,
           w_qkv, b_qkv, w_dense, b_dense):
    t0s, in_maps = _host_prep(hidden_states, alibi, attention_mask,
                              w_qkv, b_qkv, w_dense)
    if t0s not in _CACHE:
        _CACHE[t0s] = _build(t0s)
    nc = _CACHE[t0s]
    res = run_bass_kernel_spmd(nc, in_maps, list(range(NCORES)))
    acc = res.results[0]["out_part"].astype(np.float64)
    for c in range(1, NCORES):
        acc += res.results[c]["out_part"].astype(np.float64)
    acc = acc.T  # device produces [H, S]
    # v-bias folded: ctx_true = ctx + b_v  ->  + w_dense @ b_v
    bv = np.asarray(b_qkv, np.float32).reshape(NH, 3, HD)[:, 2].reshape(-1)
    extra = np.asarray(w_dense, np.float32) @ bv
    out = acc.astype(np.float32) + extra[None, :]
    out = out + np.asarray(b_dense, np.float32)[None, :]
    out = out + np.asarray(residual, np.float32).reshape(S, H)
    return out.reshape(B, S, H).astype(np.float32)
